# revision 44
# baseline (speedup 1.0000x reference)
"""FP8 dynamic-quantized linear (nn_FP8Linear) on 8 Trainium2 NeuronCores.

out = fp16((x_fp8 @ w_fp8.T) / (sx*sw)) + bias, with per-tensor dynamic
fp8-e4m3 quantization of x and weight (scale = FP8_MAX / amax).

Sharding: 2D 4x2 grid — x rows split in 4 groups of 1024, weight rows
(out_features) split in 2 groups of 2048. Per core: 8.4MB of x + 16.8MB of
weight in (25.2MB vs 50MB for 1D), out slab [1024, 2048].

The host pre-TRANSPOSES each core's slabs (k-major), so the device needs
only straight DMACopies: DmaTransposeAnt is hard-serialized against
CollectiveCompute by the Tile framework, which would freeze the input
stream for the whole amax-AllGather window; straight copies also run at
~360GB/s vs ~293GB/s for the transpose engine.

Global per-tensor amaxes: each core is amax-responsible for a disjoint
slice (one k-half of its x slab, one n-quarter of its w slab; the 8 cores'
slices tile x and w exactly). Those slices stream FIRST; partial amaxes
are reduced with per-arrival abs-max tensor_reduce on DVE and combined
with two AllGathers (15us each in the cost model vs 28.1us for AllReduce)
+ a local max. The x AllGather goes first: full x8 gates the matmul,
while w is consumed block-by-block, so x's quantize hides under w's
AllGather window.

The per-core k-axis is ROLLED (local k = global k rotated by 2048*j) so
one SPMD program serves all 8 cores; same for w's n-axis (block i first).
The host un-permutes output columns when assembling.

w8 lives in a 2-block rotating pool (PE consumes block b while block b+1
quantizes), halving its SBUF footprint so all fp16 staging fits.

TRN fp8e4 (float8_e4m3) has max +-240 vs OCP e4m3fn's +-448, so the device
uses scale 224/amax == ref_scale/2: fp8 grids are self-similar under powers
of two, so device fp8 values are exactly half the reference's, and the
dequant multipliers (= 2x the reference's each) absorb the factor of 4.
"""

import time

import numpy as np

import concourse.bacc as bacc
import concourse.bass_isa as bass_isa
import concourse.mybir as mybir
import concourse.tile as tile
from concourse.bass_utils import run_bass_kernel_spmd

F16 = mybir.dt.float16
F32 = mybir.dt.float32
F8 = mybir.dt.float8e4

NCORES = 8
P_ROWS = 4          # x row groups
Q_COLS = 2          # weight (out_feature) column groups
EPS = 1e-12
DEV_FP8_MAX = 224.0


def build_kernel(M=4096, K=4096, N=4096, double_row=True):
    """Build + compile the per-core SPMD bass program.

    Per-core inputs (host pre-sliced AND pre-transposed to k-major; k/n
    axes rolled to local order):
      xat [K/2, MI]   x amax-responsible k-half (local k 0..K/2)
      xrt [K/2, MI]   rest of x
      wat [K, NB]     w amax-responsible n-block (local n 0..NB)
      wrt [K, NJ-NB]  rest of w, blocks in stream order
      bias [1, NJ]    bias slice in local n order
    Output: out [MI, NJ] in local n order.
    """
    MI = M // P_ROWS            # 1024 rows per core
    NJ = N // Q_COLS            # 2048 out cols per core
    NBLK = 4                    # w blocks per core (amax granularity)
    NB = NJ // NBLK             # 512
    KB = K // 256               # 16 DoubleRow k-blocks
    MC = MI // 128              # 8 m-chunks
    XT = 4                      # transfers per x half   ([512, MI] each)
    WT = 4                      # transfers per w block  ([1024, NB] each)

    nc = bacc.Bacc("TRN2", target_bir_lowering=False, debug=False,
                   num_devices=NCORES)
    xat = nc.dram_tensor("xat", [K // 2, MI], F16, kind="ExternalInput").ap()
    xrt = nc.dram_tensor("xrt", [K // 2, MI], F16, kind="ExternalInput").ap()
    wat = nc.dram_tensor("wat", [K, NB], F16, kind="ExternalInput").ap()
    wrt = nc.dram_tensor("wrt", [K, NJ - NB], F16, kind="ExternalInput").ap()
    bias = nc.dram_tensor("bias", [1, NJ], F16, kind="ExternalInput").ap()
    out = nc.dram_tensor("out", [MI, NJ], F16, kind="ExternalOutput").ap()

    with tile.TileContext(nc) as tc:
        with (
            tc.tile_pool(name="const", bufs=1) as cpool,
            tc.tile_pool(name="redu", bufs=2) as rpool,
            tc.tile_pool(name="stg", bufs=12) as spool,
            tc.tile_pool(name="w8", bufs=2 * KB) as w8pool,
            tc.tile_pool(name="x8", bufs=KB) as x8pool,
            tc.tile_pool(name="ot", bufs=4) as opool,
            tc.tile_pool(name="psum", bufs=8, space="PSUM") as ppool,
            tc.tile_pool(name="dram", bufs=4, space="DRAM") as dpool,
        ):
            # ---- bias broadcast; tiny activation to pre-warm the act table
            bias_row = cpool.tile([1, NJ], F16, tag="bias_row")
            nc.gpsimd.dma_start(bias_row[:], bias[:])
            bias_b = cpool.tile([128, NJ], F16, tag="bias_b")
            nc.gpsimd.partition_broadcast(bias_b[:], bias_row[:])
            warm = rpool.tile([1, 8], F16, tag="warm", name="warm")
            nc.scalar.activation(warm[:], bias_row[0:1, 0:8],
                                 mybir.ActivationFunctionType.Copy, scale=1.0)

            # ---- fp8 destination tiles --------------------------------
            x8 = [x8pool.tile([128, 2 * MI], F8, tag="x8", name=f"x8_{kb}")
                  for kb in range(KB)]
            # w8 rotates: 2 block slots of KB tiles each
            w8 = [[w8pool.tile([128, 2 * NB], F8, tag="w8",
                               name=f"w8_{b}_{kb}") for kb in range(KB)]
                  for b in range(NBLK)]

            # ---- amax-responsible streams (straight DMAs, 1MB each) ----
            # x-amax: src xat[512t : 512(t+1), :] -> [128, 4, MI]
            xamx = []
            for t in range(XT):
                stg = spool.tile([128, 4 * MI], F16, tag="stg",
                                 name=f"xamx_{t}")
                nc.sync.dma_start(
                    stg[:].rearrange("p (c m) -> p c m", c=4),
                    xat[512 * t:512 * (t + 1), :].rearrange(
                        "(c p) m -> p c m", p=128))
                xamx.append(stg)

            def absmax_chain(tiles, tag):
                """Per-arrival abs-max tensor_reduce ([128, 4096] -> one
                column of a shared [128, XT] partials tile). No combine ops:
                partition_all_reduce handles partitions per column, and the
                max over columns folds into the existing post-AllGather
                reduce. (DVE has no 2x mode for reductions and the HW
                codegen rejects the abs_max ALU op, so per-arrival reduces
                are the fastest lowerable form.)"""
                pm = rpool.tile([128, len(tiles)], F32, tag=f"pm_{tag}",
                                name=f"pm_{tag}", bufs=1)
                for idx, stg in enumerate(tiles):
                    nc.vector.tensor_reduce(
                        pm[:, idx:idx + 1], stg[:], axis=mybir.AxisListType.X,
                        op=mybir.AluOpType.max, apply_absolute_value=True)
                return pm

            px = absmax_chain(xamx, "x")

            # w-amax: src wat[1024t : 1024(t+1), :] -> [128, 8, NB]
            wamx = []
            wamx_dmas = []
            for t in range(WT):
                stg = spool.tile([128, 8 * NB], F16, tag="stg",
                                 name=f"wamx_{t}")
                d = nc.sync.dma_start(
                    stg[:].rearrange("p (c n) -> p c n", c=8),
                    wat[1024 * t:1024 * (t + 1), :].rearrange(
                        "(c p) n -> p c n", p=128))
                wamx.append(stg)
                wamx_dmas.append(d)
            pw = absmax_chain(wamx, "w")

            # ---- global amax: per-tensor AllGather(8x1) + local max ----
            # Fire both gathers first (Pool queue order matters: a readback
            # DMA waiting on AllGather-x would head-of-line-block the
            # AllGather-w issue), then collect both.
            def fire_gather(part, tag, np_=XT):
                pr = rpool.tile([128, np_], F32, tag=f"pr_{tag}",
                                name=f"pr_{tag}")
                nc.gpsimd.partition_all_reduce(
                    pr[:], part[:], channels=128,
                    reduce_op=bass_isa.ReduceOp.max)
                bin_ = dpool.tile([1, np_], F32, name=f"bin_{tag}")
                bout = dpool.tile([NCORES, np_], F32, name=f"bout_{tag}")
                bdma = nc.gpsimd.dma_start(bin_[:], pr[0:1, :])
                nc.gpsimd.collective_compute(
                    "AllGather", mybir.AluOpType.bypass,
                    replica_groups=[list(range(NCORES))],
                    ins=[bin_.opt()], outs=[bout.opt()])
                return bout, bdma

            def collect_scale(bout, tag, np_=XT):
                nall = NCORES * np_
                rall = rpool.tile([1, nall], F32, tag=f"rall_{tag}",
                                  name=f"rall_{tag}")
                rdma = nc.gpsimd.dma_start(
                    rall[:],
                    bout.opt().rearrange("c o -> (c o)").rearrange(
                        "(o f) -> o f", o=1))
                collect_scale.last_rdma = rdma
                g = rpool.tile([1, 1], F32, tag=f"g_{tag}", name=f"g_{tag}")
                nc.vector.tensor_reduce(
                    g[:], rall[:], axis=mybir.AxisListType.X,
                    op=mybir.AluOpType.max)
                # inv = max(amax, EPS)/224 (the dequant multiplier);
                # s = 1/inv. All math on [1,1], then ONE broadcast (a
                # mid-chain broadcast costs two extra DVE<->Pool sem hops).
                pair = rpool.tile([1, 2], F32, tag=f"pair_{tag}",
                                  name=f"pair_{tag}")
                nc.vector.tensor_scalar(
                    pair[:, 0:1], g[:], EPS, 1.0 / DEV_FP8_MAX,
                    op0=mybir.AluOpType.max, op1=mybir.AluOpType.mult)
                nc.vector.reciprocal(pair[:, 1:2], pair[:, 0:1])
                pairb = rpool.tile([128, 2], F32, tag=f"pairb_{tag}",
                                   name=f"pairb_{tag}")
                nc.gpsimd.partition_broadcast(pairb[:], pair[:])
                return pairb[:, 1:2], pairb[:, 0:1]

            bout_x, binx_dma = fire_gather(px, "x")
            bout_w, binw_dma = fire_gather(pw, "w")
            # x scale only here: emitting the w-scale DVE math now would
            # head-of-line block the x8 quantize ops (which only need sx)
            # in DVE's in-order queue until the second AllGather returns.
            sx, inv_sx = collect_scale(bout_x, "x")

            from concourse.bass import _add_dep_helper

            def _inst(h):
                return getattr(h, "ins", h)

            # ---- x rest stream ----------------------------------------
            xrest = []
            xrest_dmas = []
            for t in range(XT):
                stg = spool.tile([128, 4 * MI], F16, tag="stg",
                                 name=f"xrest_{t}")
                d = nc.sync.dma_start(
                    stg[:].rearrange("p (c m) -> p c m", c=4),
                    xrt[512 * t:512 * (t + 1), :].rearrange(
                        "(c p) m -> p c m", p=128))
                xrest.append(stg)
                xrest_dmas.append(d)
            # The DMA engine is FIFO by descriptor-gen completion; without
            # this edge the xrest stream enters the queue at t=0 and the
            # tiny AllGather-input DMA (critical path) waits ~12us behind it.
            _add_dep_helper(_inst(xrest_dmas[0]), _inst(binx_dma), sync=True,
                            reason="yield DMA slot to amax gather input")
            # The last w-amax transfer also yields: the w reduces are
            # DVE-throughput-bound, not arrival-bound, so delaying it ~3us
            # is free while the x gather input gets its slot sooner.
            _add_dep_helper(_inst(wamx_dmas[3]), _inst(binx_dma), sync=True,
                            reason="yield DMA slot to amax gather input")
            # (binw_dma needs no yield edge: its slot falls in the natural
            # DMA gap between the xrest stream and the stg-WAR-blocked wr.)

            # ---- quantize helpers (static engine assignment; only DVE
            # and Act: gpsimd elementwise ops fail the HW engine check) ----
            def quant(eng, dst, src, s_ap):
                if eng == "V":
                    return nc.vector.tensor_scalar(
                        dst, src, s_ap, None, op0=mybir.AluOpType.mult)
                return nc.scalar.activation(
                    dst, src, mybir.ActivationFunctionType.Copy, scale=s_ap)

            # x8 quantize: 32 slices [128, MI]; local chunk cg=16h+4t+v:
            # kb = cg//2, half = cg%2. V20/A12 split matches the engines'
            # 1.72 vs 0.99 elem/ns rates.
            X_ENG = ["V", "A", "V", "V", "V", "A", "V", "A"]
            for h, tiles in ((0, xamx), (1, xrest)):
                for t in range(XT):
                    stg = tiles[t]
                    for v in range(4):
                        cg = h * 16 + t * 4 + v
                        kb, hh = cg // 2, cg % 2
                        dst = x8[kb][:, hh * MI:(hh + 1) * MI]
                        quant(X_ENG[(t % 2) * 4 + v], dst,
                              stg[:, v * MI:(v + 1) * MI], sx)

            sw, inv_sw = collect_scale(bout_w, "w")
            rallw_dma = collect_scale.last_rdma
            rxw = rpool.tile([128, 1], F32, tag="rxw", name="rxw")
            nc.vector.tensor_tensor(rxw[:], inv_sx, inv_sw,
                                    op=mybir.AluOpType.mult)

            # w8 block quantize: 16 ops [128, 2, NB] per block (one full
            # w8 tile each). Block 0 gates PE start: balance V/A.
            W0_ENG = ["V", "A", "V", "A", "V", "A", "V", "V",
                      "A", "V", "A", "V", "V", "A", "V", "V"]
            WR_ENG = ["V", "A", "V", "V"] * 4

            def quant_wblock(b, tiles, engs):
                insts = []
                for t in range(WT):
                    stg3 = tiles[t][:].rearrange("p (c n) -> p c n", c=8)
                    for q in range(4):
                        kb = t * 4 + q
                        dst = w8[b][kb].rearrange("p (i n) -> p i n", i=2)
                        insts.append(quant(engs[t * 4 + q], dst,
                                           stg3[:, 2 * q:2 * q + 2, :], sw))
                return insts

            w0_insts = quant_wblock(0, wamx, W0_ENG)

            # ---- w rest: stream blocks 1..3 (DMA issues up front; the
            # quantize ops are emitted inside the matmul loop so each
            # block's epilog bias-adds aren't queued behind later blocks'
            # quantizes in DVE's in-order stream) -----------------------
            wrtiles = {}
            wr_dmas = {}
            for b in range(1, NBLK):
                tiles = []
                for t in range(WT):
                    stg = spool.tile([128, 8 * NB], F16, tag="stg",
                                     name=f"wr_{b}_{t}")
                    d = nc.sync.dma_start(
                        stg[:].rearrange("p (c n) -> p c n", c=8),
                        wrt[1024 * t:1024 * (t + 1),
                            (b - 1) * NB:b * NB].rearrange(
                            "(c p) n -> p c n", p=128))
                    tiles.append(stg)
                    wr_dmas[(b, t)] = d
                wrtiles[b] = tiles
            # Blocks 2-3 aren't needed for ~25us — keep them out of the DMA
            # queue until the w-scale readback (critical path) has its slot.
            for b, t in [(1, 3)] + [(b, t) for b in (2, 3)
                                    for t in range(WT)]:
                _add_dep_helper(_inst(wr_dmas[(b, t)]), _inst(rallw_dma),
                                sync=True,
                                reason="yield DMA slot to scale readback")

            # ---- matmul: block-outer (w stream order), mc inner --------
            first_mm = None
            for b in range(NBLK):
                if b + 1 < NBLK:
                    quant_wblock(b + 1, wrtiles[b + 1], WR_ENG)
                for mc in range(MC):
                    ps = ppool.tile([128, NB], F32, tag="ps",
                                    name=f"ps_{b}_{mc}")
                    if double_row:
                        for kb in range(KB):
                            lhsT = x8[kb].rearrange(
                                "p (i m) -> p i m", i=2)[
                                :, :, mc * 128:(mc + 1) * 128]
                            rhs = w8[b][kb].rearrange("p (i n) -> p i n", i=2)
                            mm = nc.tensor.matmul(
                                ps[:], lhsT, rhs,
                                start=(kb == 0), stop=(kb == KB - 1),
                                perf_mode=mybir.MatmulPerfMode.DoubleRow)
                            if first_mm is None:
                                first_mm = mm
                    else:
                        for kb in range(KB):
                            for i in range(2):
                                lhsT = x8[kb][:, i * MI + mc * 128:
                                              i * MI + (mc + 1) * 128]
                                rhs = w8[b][kb][:, i * NB:(i + 1) * NB]
                                nc.tensor.matmul(
                                    ps[:], lhsT, rhs,
                                    start=(kb == 0 and i == 0),
                                    stop=(kb == KB - 1 and i == 1))
                    # epilog: Act dequant psum->f16 tile, DVE bias add,
                    # out DMA issued from SP (idle after the input streams)
                    ot = opool.tile([128, NB], F16, tag="ot",
                                    name=f"ot_{b}_{mc}")
                    nc.scalar.activation(ot[:], ps[:],
                                         mybir.ActivationFunctionType.Copy,
                                         scale=rxw[:])
                    nc.vector.tensor_tensor(
                        ot[:], ot[:], bias_b[:, b * NB:(b + 1) * NB],
                        op=mybir.AluOpType.add)
                    m0 = mc * 128
                    nc.sync.dma_start(
                        out[m0:m0 + 128, b * NB:(b + 1) * NB], ot[:])

            # Start the PE stream only once block 0 is fully quantized:
            # chasing the quantize per-kb stalls PE repeatedly, and every
            # stall resets the clock ramp (~2us of slow matmuls each).
            lastV = max(i for i, e in enumerate(W0_ENG) if e == "V")
            lastA = max(i for i, e in enumerate(W0_ENG) if e == "A")
            for qi in (lastV, lastA):
                _add_dep_helper(_inst(first_mm), _inst(w0_insts[qi]),
                                sync=True, reason="clean PE ramp")

    nc.compile()
    return nc


DOUBLE_ROW = True

_CACHE = {}


def _get_kernel(M=4096, K=4096, NSH=512, SW=2048, double_row=None):
    """Signature kept compatible with the previous 1D kernel; NSH/SW are
    ignored (sharding is the fixed 4x2 grid)."""
    if double_row is None:
        double_row = DOUBLE_ROW
    N = NSH * NCORES if NSH else M
    key = (M, K, N, double_row)
    if key not in _CACHE:
        _CACHE[key] = build_kernel(M, K, N, double_row)
    return _CACHE[key]


def kernel(x, weight, bias):
    M, K = x.shape
    N = weight.shape[0]
    nc = _get_kernel(M, K, N // NCORES, 2048)
    MI = M // P_ROWS
    NJ = N // Q_COLS
    NB = NJ // 4

    x = np.asarray(x)
    weight = np.asarray(weight)
    bias = np.asarray(bias)
    in_maps = []
    for c in range(NCORES):
        i, j = c // Q_COLS, c % Q_COLS
        # local k order: global k rolled by 2048*j (amax half first)
        k0 = (K // Q_COLS) * j
        xs = x[i * MI:(i + 1) * MI, :]
        xat = xs[:, k0:k0 + K // 2].T
        if k0:
            xrt_ = np.concatenate(
                [xs[:, k0 + K // 2:], xs[:, :k0]], axis=1).T
        else:
            xrt_ = xs[:, K // 2:].T
        ws = weight[j * NJ:(j + 1) * NJ, :]
        wsr = np.concatenate([ws[:, k0:], ws[:, :k0]], axis=1) if k0 else ws
        # local n order: block i first, then i+1.. (mod 4)
        border = [(i + g) % 4 for g in range(4)]
        wat = wsr[border[0] * NB:(border[0] + 1) * NB, :].T
        wrt_ = np.concatenate(
            [wsr[bb * NB:(bb + 1) * NB, :] for bb in border[1:]], axis=0).T
        bs = bias[j * NJ:(j + 1) * NJ]
        bl = np.concatenate([bs[bb * NB:(bb + 1) * NB] for bb in border])
        in_maps.append({
            "xat": np.ascontiguousarray(xat),
            "xrt": np.ascontiguousarray(xrt_),
            "wat": np.ascontiguousarray(wat),
            "wrt": np.ascontiguousarray(wrt_),
            "bias": np.ascontiguousarray(bl.reshape(1, NJ)),
        })
    # The axon terminal occasionally reports a stale NRT_EXEC_UNIT error from
    # a previous session on first use; a retry lands on a recovered device.
    last_err = None
    for _ in range(3):
        try:
            res = run_bass_kernel_spmd(nc, in_maps,
                                       core_ids=list(range(NCORES)))
            break
        except Exception as e:  # noqa: BLE001
            last_err = e
            time.sleep(2.0)
    else:
        raise last_err
    full = np.empty((M, N), dtype=np.float16)
    for c in range(NCORES):
        i, j = c // Q_COLS, c % Q_COLS
        oc = res.results[c]["out"]
        # un-permute local n blocks back to global order
        for g in range(4):
            bb = (i + g) % 4
            full[i * MI:(i + 1) * MI,
                 j * NJ + bb * NB:j * NJ + (bb + 1) * NB] = \
                oc[:, g * NB:(g + 1) * NB]
    return full


# revision 52
# speedup vs baseline: 1.0089x; 1.0089x over previous
"""FP8 dynamic-quantized linear (nn_FP8Linear) on 8 Trainium2 NeuronCores.

out = fp16((x_fp8 @ w_fp8.T) / (sx*sw)) + bias, with per-tensor dynamic
fp8-e4m3 quantization of x and weight (scale = FP8_MAX / amax).

Sharding: 2D 4x2 grid — x rows split in 4 groups of 1024, weight rows
(out_features) split in 2 groups of 2048. Per core: 8.4MB of x + 16.8MB of
weight in (25.2MB vs 50MB for 1D), out slab [1024, 2048].

The host pre-TRANSPOSES each core's slabs (k-major), so the device needs
only straight DMACopies: DmaTransposeAnt is hard-serialized against
CollectiveCompute by the Tile framework, which would freeze the input
stream for the whole amax-AllGather window; straight copies also run at
~360GB/s vs ~293GB/s for the transpose engine.

Global per-tensor amaxes: each core is amax-responsible for a disjoint
slice (one k-half of its x slab, one n-quarter of its w slab; the 8 cores'
slices tile x and w exactly). Those slices stream FIRST; partial amaxes
are reduced with per-arrival abs-max tensor_reduce on DVE and combined
with two AllGathers (15us each in the cost model vs 28.1us for AllReduce)
+ a local max. The x AllGather goes first: full x8 gates the matmul,
while w is consumed block-by-block, so x's quantize hides under w's
AllGather window.

The per-core k-axis is ROLLED (local k = global k rotated by 2048*j) so
one SPMD program serves all 8 cores; same for w's n-axis (block i first).
The host un-permutes output columns when assembling.

w8 lives in a 2-block rotating pool (PE consumes block b while block b+1
quantizes), halving its SBUF footprint so all fp16 staging fits.

TRN fp8e4 (float8_e4m3) has max +-240 vs OCP e4m3fn's +-448, so the device
uses scale 224/amax == ref_scale/2: fp8 grids are self-similar under powers
of two, so device fp8 values are exactly half the reference's, and the
dequant multipliers (= 2x the reference's each) absorb the factor of 4.
"""

import time

import numpy as np

import concourse.bacc as bacc
import concourse.bass_isa as bass_isa
import concourse.mybir as mybir
import concourse.tile as tile
from concourse.bass_utils import run_bass_kernel_spmd

F16 = mybir.dt.float16
F32 = mybir.dt.float32
F8 = mybir.dt.float8e4

NCORES = 8
P_ROWS = 4          # x row groups
Q_COLS = 2          # weight (out_feature) column groups
EPS = 1e-12
DEV_FP8_MAX = 224.0


def build_kernel(M=4096, K=4096, N=4096, double_row=True):
    """Build + compile the per-core SPMD bass program.

    Per-core inputs (host pre-sliced AND pre-transposed to k-major; k/n
    axes rolled to local order):
      xat [K/2, MI]   x amax-responsible k-half (local k 0..K/2)
      xrt [K/2, MI]   rest of x
      wat [K, NB]     w amax-responsible n-block (local n 0..NB)
      wrt [K, NJ-NB]  rest of w, blocks in stream order
      bias [1, NJ]    bias slice in local n order
    Output: out [MI, NJ] in local n order.
    """
    MI = M // P_ROWS            # 1024 rows per core
    NJ = N // Q_COLS            # 2048 out cols per core
    NBLK = 4                    # w blocks per core (amax granularity)
    NB = NJ // NBLK             # 512
    KB = K // 256               # 16 DoubleRow k-blocks
    MC = MI // 128              # 8 m-chunks
    XT = 4                      # transfers per x half   ([512, MI] each)
    WT = 4                      # transfers per w block  ([1024, NB] each)

    nc = bacc.Bacc("TRN2", target_bir_lowering=False, debug=False,
                   num_devices=NCORES)
    xat = nc.dram_tensor("xat", [K // 2, MI], F16, kind="ExternalInput").ap()
    xrt = nc.dram_tensor("xrt", [K // 2, MI], F16, kind="ExternalInput").ap()
    wat = nc.dram_tensor("wat", [K, NB], F16, kind="ExternalInput").ap()
    wrt = nc.dram_tensor("wrt", [K, NJ - NB], F16, kind="ExternalInput").ap()
    bias = nc.dram_tensor("bias", [1, NJ], F16, kind="ExternalInput").ap()
    out = nc.dram_tensor("out", [MI, NJ], F16, kind="ExternalOutput").ap()

    with tile.TileContext(nc) as tc:
        with (
            tc.tile_pool(name="const", bufs=1) as cpool,
            tc.tile_pool(name="redu", bufs=2) as rpool,
            tc.tile_pool(name="stg", bufs=12) as spool,
            tc.tile_pool(name="w8", bufs=2 * KB) as w8pool,
            tc.tile_pool(name="x8", bufs=KB) as x8pool,
            tc.tile_pool(name="ot", bufs=4) as opool,
            tc.tile_pool(name="psum", bufs=8, space="PSUM") as ppool,
            tc.tile_pool(name="dram", bufs=4, space="DRAM") as dpool,
        ):
            # ---- bias broadcast; tiny activation to pre-warm the act table
            bias_row = cpool.tile([1, NJ], F16, tag="bias_row")
            nc.gpsimd.dma_start(bias_row[:], bias[:])
            bias_b = cpool.tile([128, NJ], F16, tag="bias_b")
            nc.gpsimd.partition_broadcast(bias_b[:], bias_row[:])
            warm = rpool.tile([1, 8], F16, tag="warm", name="warm")
            nc.scalar.activation(warm[:], bias_row[0:1, 0:8],
                                 mybir.ActivationFunctionType.Copy, scale=1.0)

            # ---- fp8 destination tiles --------------------------------
            x8 = [x8pool.tile([128, 2 * MI], F8, tag="x8", name=f"x8_{kb}")
                  for kb in range(KB)]
            # w8 rotates: 2 block slots of KB tiles each
            w8 = [[w8pool.tile([128, 2 * NB], F8, tag="w8",
                               name=f"w8_{b}_{kb}") for kb in range(KB)]
                  for b in range(NBLK)]

            # ---- amax-responsible streams (straight DMAs, 1MB each) ----
            # x-amax: src xat[512t : 512(t+1), :] -> [128, 4, MI]
            xamx = []
            for t in range(XT):
                stg = spool.tile([128, 4 * MI], F16, tag="stg",
                                 name=f"xamx_{t}")
                nc.sync.dma_start(
                    stg[:].rearrange("p (c m) -> p c m", c=4),
                    xat[512 * t:512 * (t + 1), :].rearrange(
                        "(c p) m -> p c m", p=128))
                xamx.append(stg)

            def absmax_chain(tiles, tag):
                """Per-arrival abs-max tensor_reduce ([128, 4096] -> one
                column of a shared [128, XT] partials tile). No combine ops:
                partition_all_reduce handles partitions per column, and the
                max over columns folds into the existing post-AllGather
                reduce. (DVE has no 2x mode for reductions and the HW
                codegen rejects the abs_max ALU op, so per-arrival reduces
                are the fastest lowerable form.)"""
                pm = rpool.tile([128, len(tiles)], F32, tag=f"pm_{tag}",
                                name=f"pm_{tag}", bufs=1)
                for idx, stg in enumerate(tiles):
                    nc.vector.tensor_reduce(
                        pm[:, idx:idx + 1], stg[:], axis=mybir.AxisListType.X,
                        op=mybir.AluOpType.max, apply_absolute_value=True)
                return pm

            px = absmax_chain(xamx, "x")

            # w-amax: src wat[1024t : 1024(t+1), :] -> [128, 8, NB]
            wamx = []
            wamx_dmas = []
            for t in range(WT):
                stg = spool.tile([128, 8 * NB], F16, tag="stg",
                                 name=f"wamx_{t}")
                d = nc.sync.dma_start(
                    stg[:].rearrange("p (c n) -> p c n", c=8),
                    wat[1024 * t:1024 * (t + 1), :].rearrange(
                        "(c p) n -> p c n", p=128))
                wamx.append(stg)
                wamx_dmas.append(d)
            pw = absmax_chain(wamx, "w")

            # ---- global amax: per-tensor AllGather(8x1) + local max ----
            # Fire both gathers first (Pool queue order matters: a readback
            # DMA waiting on AllGather-x would head-of-line-block the
            # AllGather-w issue), then collect both.
            def fire_gather(part, tag, np_=XT):
                pr = rpool.tile([128, np_], F32, tag=f"pr_{tag}",
                                name=f"pr_{tag}")
                nc.gpsimd.partition_all_reduce(
                    pr[:], part[:], channels=128,
                    reduce_op=bass_isa.ReduceOp.max)
                bin_ = dpool.tile([1, np_], F32, name=f"bin_{tag}")
                bout = dpool.tile([NCORES, np_], F32, name=f"bout_{tag}")
                bdma = nc.gpsimd.dma_start(bin_[:], pr[0:1, :])
                nc.gpsimd.collective_compute(
                    "AllGather", mybir.AluOpType.bypass,
                    replica_groups=[list(range(NCORES))],
                    ins=[bin_.opt()], outs=[bout.opt()])
                return bout, bdma

            def collect_scale(bout, tag, np_=XT):
                nall = NCORES * np_
                rall = rpool.tile([1, nall], F32, tag=f"rall_{tag}",
                                  name=f"rall_{tag}")
                rdma = nc.gpsimd.dma_start(
                    rall[:],
                    bout.opt().rearrange("c o -> (c o)").rearrange(
                        "(o f) -> o f", o=1))
                collect_scale.last_rdma = rdma
                g = rpool.tile([1, 1], F32, tag=f"g_{tag}", name=f"g_{tag}")
                nc.vector.tensor_reduce(
                    g[:], rall[:], axis=mybir.AxisListType.X,
                    op=mybir.AluOpType.max)
                # inv = max(amax, EPS)/224 (the dequant multiplier);
                # s = 1/inv. All math on [1,1], then ONE broadcast (a
                # mid-chain broadcast costs two extra DVE<->Pool sem hops).
                pair = rpool.tile([1, 2], F32, tag=f"pair_{tag}",
                                  name=f"pair_{tag}")
                nc.vector.tensor_scalar(
                    pair[:, 0:1], g[:], EPS, 1.0 / DEV_FP8_MAX,
                    op0=mybir.AluOpType.max, op1=mybir.AluOpType.mult)
                nc.vector.reciprocal(pair[:, 1:2], pair[:, 0:1])
                pairb = rpool.tile([128, 2], F32, tag=f"pairb_{tag}",
                                   name=f"pairb_{tag}")
                nc.gpsimd.partition_broadcast(pairb[:], pair[:])
                return pairb[:, 1:2], pairb[:, 0:1]

            bout_x, binx_dma = fire_gather(px, "x")
            bout_w, binw_dma = fire_gather(pw, "w")
            # x scale only here: emitting the w-scale DVE math now would
            # head-of-line block the x8 quantize ops (which only need sx)
            # in DVE's in-order queue until the second AllGather returns.
            sx, inv_sx = collect_scale(bout_x, "x")

            from concourse.bass import _add_dep_helper

            def _inst(h):
                return getattr(h, "ins", h)

            # ---- x rest stream ----------------------------------------
            xrest = []
            xrest_dmas = []
            for t in range(XT):
                stg = spool.tile([128, 4 * MI], F16, tag="stg",
                                 name=f"xrest_{t}")
                d = nc.sync.dma_start(
                    stg[:].rearrange("p (c m) -> p c m", c=4),
                    xrt[512 * t:512 * (t + 1), :].rearrange(
                        "(c p) m -> p c m", p=128))
                xrest.append(stg)
                xrest_dmas.append(d)
            # The DMA engine is FIFO by descriptor-gen completion; without
            # this edge the xrest stream enters the queue at t=0 and the
            # tiny AllGather-input DMA (critical path) waits ~12us behind it.
            _add_dep_helper(_inst(xrest_dmas[0]), _inst(binx_dma), sync=True,
                            reason="yield DMA slot to amax gather input")
            # The last w-amax transfer also yields: the w reduces are
            # DVE-throughput-bound, not arrival-bound, so delaying it ~3us
            # is free while the x gather input gets its slot sooner.
            _add_dep_helper(_inst(wamx_dmas[3]), _inst(binx_dma), sync=True,
                            reason="yield DMA slot to amax gather input")
            # (binw_dma needs no yield edge: its slot falls in the natural
            # DMA gap between the xrest stream and the stg-WAR-blocked wr.)

            # ---- quantize helpers (static engine assignment; only DVE
            # and Act: gpsimd elementwise ops fail the HW engine check) ----
            def quant(eng, dst, src, s_ap):
                if eng == "V":
                    return nc.vector.tensor_scalar(
                        dst, src, s_ap, None, op0=mybir.AluOpType.mult)
                return nc.scalar.activation(
                    dst, src, mybir.ActivationFunctionType.Copy, scale=s_ap)

            # x8 quantize: 32 slices [128, MI]; local chunk cg=16h+4t+v:
            # kb = cg//2, half = cg%2. V20/A12 split matches the engines'
            # 1.72 vs 0.99 elem/ns rates.
            X_ENG = ["V", "A", "V", "V", "V", "A", "V", "A"]
            for h, tiles in ((0, xamx), (1, xrest)):
                for t in range(XT):
                    stg = tiles[t]
                    for v in range(4):
                        cg = h * 16 + t * 4 + v
                        kb, hh = cg // 2, cg % 2
                        dst = x8[kb][:, hh * MI:(hh + 1) * MI]
                        quant(X_ENG[(t % 2) * 4 + v], dst,
                              stg[:, v * MI:(v + 1) * MI], sx)

            sw, inv_sw = collect_scale(bout_w, "w")
            rallw_dma = collect_scale.last_rdma
            rxw = rpool.tile([128, 1], F32, tag="rxw", name="rxw")
            nc.vector.tensor_tensor(rxw[:], inv_sx, inv_sw,
                                    op=mybir.AluOpType.mult)

            # w8 block quantize: 16 ops [128, 2, NB] per block (one full
            # w8 tile each). Block 0 gates PE start: balance V/A.
            W0_ENG = ["V", "A", "V", "A", "V", "A", "V", "V",
                      "A", "V", "A", "V", "V", "A", "V", "V"]
            WR_ENG = ["V", "A", "V", "V"] * 4

            def quant_wblock(b, tiles, engs):
                insts = []
                for t in range(WT):
                    stg3 = tiles[t][:].rearrange("p (c n) -> p c n", c=8)
                    for q in range(4):
                        kb = t * 4 + q
                        dst = w8[b][kb].rearrange("p (i n) -> p i n", i=2)
                        insts.append(quant(engs[t * 4 + q], dst,
                                           stg3[:, 2 * q:2 * q + 2, :], sw))
                return insts

            w0_insts = quant_wblock(0, wamx, W0_ENG)

            # ---- w rest: stream blocks 1..3 (DMA issues up front; the
            # quantize ops are emitted inside the matmul loop so each
            # block's epilog bias-adds aren't queued behind later blocks'
            # quantizes in DVE's in-order stream) -----------------------
            wrtiles = {}
            wr_dmas = {}
            for b in range(1, NBLK):
                tiles = []
                for t in range(WT):
                    stg = spool.tile([128, 8 * NB], F16, tag="stg",
                                     name=f"wr_{b}_{t}")
                    d = nc.sync.dma_start(
                        stg[:].rearrange("p (c n) -> p c n", c=8),
                        wrt[1024 * t:1024 * (t + 1),
                            (b - 1) * NB:b * NB].rearrange(
                            "(c p) n -> p c n", p=128))
                    tiles.append(stg)
                    wr_dmas[(b, t)] = d
                wrtiles[b] = tiles
            # Blocks 2-3 aren't needed for ~25us — keep them out of the DMA
            # queue until the w-scale readback (critical path) has its slot.
            for b, t in [(1, 3)] + [(b, t) for b in (2, 3)
                                    for t in range(WT)]:
                _add_dep_helper(_inst(wr_dmas[(b, t)]), _inst(rallw_dma),
                                sync=True,
                                reason="yield DMA slot to scale readback")

            # ---- matmul: block-outer (w stream order), mc inner --------
            # Block 0 runs as two 8-kb phases over its 8 resident psums:
            # phase A starts once kb0-7 are quantized (~half the block-0
            # quantize latency off the critical path) and outlasts the
            # kb8-15 quantize, so phase B never stalls.
            def chain(ps, b, mc, kb_lo, kb_hi, start, stop):
                first = None
                if double_row:
                    for kb in range(kb_lo, kb_hi):
                        lhsT = x8[kb].rearrange(
                            "p (i m) -> p i m", i=2)[
                            :, :, mc * 128:(mc + 1) * 128]
                        rhs = w8[b][kb].rearrange("p (i n) -> p i n", i=2)
                        mm = nc.tensor.matmul(
                            ps[:], lhsT, rhs,
                            start=(start and kb == kb_lo),
                            stop=(stop and kb == kb_hi - 1),
                            perf_mode=mybir.MatmulPerfMode.DoubleRow)
                        if first is None:
                            first = mm
                else:
                    for kb in range(kb_lo, kb_hi):
                        for i in range(2):
                            lhsT = x8[kb][:, i * MI + mc * 128:
                                          i * MI + (mc + 1) * 128]
                            rhs = w8[b][kb][:, i * NB:(i + 1) * NB]
                            mm = nc.tensor.matmul(
                                ps[:], lhsT, rhs,
                                start=(start and kb == kb_lo and i == 0),
                                stop=(stop and kb == kb_hi - 1 and i == 1))
                            if first is None:
                                first = mm
                return first

            def epilog(ps, b, mc):
                # Act dequant psum->f16 tile, DVE bias add, out DMA issued
                # from SP (idle after the input streams)
                ot = opool.tile([128, NB], F16, tag="ot",
                                name=f"ot_{b}_{mc}")
                nc.scalar.activation(ot[:], ps[:],
                                     mybir.ActivationFunctionType.Copy,
                                     scale=rxw[:])
                nc.vector.tensor_tensor(
                    ot[:], ot[:], bias_b[:, b * NB:(b + 1) * NB],
                    op=mybir.AluOpType.add)
                m0 = mc * 128
                nc.sync.dma_start(
                    out[m0:m0 + 128, b * NB:(b + 1) * NB], ot[:])

            # Later blocks' quantizes are emitted BEFORE earlier blocks'
            # epilogs: the epilogs have ~a full block-pass of slack, while
            # a late block-b quantize stalls the PE stream directly.
            quant_wblock(1, wrtiles[1], WR_ENG)
            ps0 = [ppool.tile([128, NB], F32, tag="ps", name=f"ps_0_{mc}")
                   for mc in range(MC)]
            first_mm = None
            for mc in range(MC):
                mm = chain(ps0[mc], 0, mc, 0, KB // 2, True, False)
                if first_mm is None:
                    first_mm = mm
            quant_wblock(2, wrtiles[2], WR_ENG)
            for mc in range(MC):
                chain(ps0[mc], 0, mc, KB // 2, KB, False, True)
                epilog(ps0[mc], 0, mc)
            quant_wblock(3, wrtiles[3], WR_ENG)
            for b in range(1, NBLK):
                for mc in range(MC):
                    ps = ppool.tile([128, NB], F32, tag="ps",
                                    name=f"ps_{b}_{mc}")
                    chain(ps, b, mc, 0, KB, True, True)
                    epilog(ps, b, mc)

            # Start the PE stream only once kb0-7 of block 0 are quantized:
            # chasing the quantize per-kb stalls PE repeatedly, and every
            # stall resets the clock ramp (~2us of slow matmuls each).
            lastV = max(i for i, e in enumerate(W0_ENG[:8]) if e == "V")
            lastA = max(i for i, e in enumerate(W0_ENG[:8]) if e == "A")
            for qi in (lastV, lastA):
                _add_dep_helper(_inst(first_mm), _inst(w0_insts[qi]),
                                sync=True, reason="clean PE ramp")

    nc.compile()
    return nc


DOUBLE_ROW = True

_CACHE = {}


def _get_kernel(M=4096, K=4096, NSH=512, SW=2048, double_row=None):
    """Signature kept compatible with the previous 1D kernel; NSH/SW are
    ignored (sharding is the fixed 4x2 grid)."""
    if double_row is None:
        double_row = DOUBLE_ROW
    N = NSH * NCORES if NSH else M
    key = (M, K, N, double_row)
    if key not in _CACHE:
        _CACHE[key] = build_kernel(M, K, N, double_row)
    return _CACHE[key]


def kernel(x, weight, bias):
    M, K = x.shape
    N = weight.shape[0]
    nc = _get_kernel(M, K, N // NCORES, 2048)
    MI = M // P_ROWS
    NJ = N // Q_COLS
    NB = NJ // 4

    x = np.asarray(x)
    weight = np.asarray(weight)
    bias = np.asarray(bias)
    in_maps = []
    for c in range(NCORES):
        i, j = c // Q_COLS, c % Q_COLS
        # local k order: global k rolled by 2048*j (amax half first)
        k0 = (K // Q_COLS) * j
        xs = x[i * MI:(i + 1) * MI, :]
        xat = xs[:, k0:k0 + K // 2].T
        if k0:
            xrt_ = np.concatenate(
                [xs[:, k0 + K // 2:], xs[:, :k0]], axis=1).T
        else:
            xrt_ = xs[:, K // 2:].T
        ws = weight[j * NJ:(j + 1) * NJ, :]
        wsr = np.concatenate([ws[:, k0:], ws[:, :k0]], axis=1) if k0 else ws
        # local n order: block i first, then i+1.. (mod 4)
        border = [(i + g) % 4 for g in range(4)]
        wat = wsr[border[0] * NB:(border[0] + 1) * NB, :].T
        wrt_ = np.concatenate(
            [wsr[bb * NB:(bb + 1) * NB, :] for bb in border[1:]], axis=0).T
        bs = bias[j * NJ:(j + 1) * NJ]
        bl = np.concatenate([bs[bb * NB:(bb + 1) * NB] for bb in border])
        in_maps.append({
            "xat": np.ascontiguousarray(xat),
            "xrt": np.ascontiguousarray(xrt_),
            "wat": np.ascontiguousarray(wat),
            "wrt": np.ascontiguousarray(wrt_),
            "bias": np.ascontiguousarray(bl.reshape(1, NJ)),
        })
    # The axon terminal occasionally reports a stale NRT_EXEC_UNIT error from
    # a previous session on first use; a retry lands on a recovered device.
    last_err = None
    for _ in range(3):
        try:
            res = run_bass_kernel_spmd(nc, in_maps,
                                       core_ids=list(range(NCORES)))
            break
        except Exception as e:  # noqa: BLE001
            last_err = e
            time.sleep(2.0)
    else:
        raise last_err
    full = np.empty((M, N), dtype=np.float16)
    for c in range(NCORES):
        i, j = c // Q_COLS, c % Q_COLS
        oc = res.results[c]["out"]
        # un-permute local n blocks back to global order
        for g in range(4):
            bb = (i + g) % 4
            full[i * MI:(i + 1) * MI,
                 j * NJ + bb * NB:j * NJ + (bb + 1) * NB] = \
                oc[:, g * NB:(g + 1) * NB]
    return full
